# revision 3
# baseline (speedup 1.0000x reference)
"""Weighted BCE loss (nn_BCELoss_with_weight) on 8 Trainium2 NeuronCores.

Reference computes:
    log_p   = max(log(pred), -100)            # clamp never binds: pred in [1e-4, 1-1e-4]
    log_1mp = max(log1p(-pred), -100)
    bce     = -(true*log_p + (1-true)*log_1mp)    # [B,C,D,H,W] = [2,16,64,128,128]
    per_class = mean(bce, axes=(0,2,3,4))         # [C]
    out = sum(weight*per_class) / sum(weight)     # scalar

Sharding: D=64 split into 8 slices of 8 (data parallel). Per core the shard
[2,16,8,128,128] is viewed as [B=2, (C,Dl)=128, H*W=16384]: partition p holds
class c=p//8 only, so per-class weighting reduces to a per-partition weighted
sum that the HOST applies to the tiny [128] per-partition totals.

Per core on device (term = t*u + (1-t)*v, u=ln(p), v=ln(1-p)):
    ACT: u = Ln(p);  v = Ln(-p + 1)                       (2 table-evals/elem)
    DVE: tensor_tensor_reduce  -> acc_tu[col] = sum(t*u)  (1 op/elem)
    DVE: scalar_tensor_tensor  -> acc_w[col]  = sum((t-1)*v)
    totals[p] = sum_cols acc_tu - sum_cols acc_w = sum_e term_e
Host: result = sum_{core,p} (-w[p//8]/(M*sum(w))) * totals[core,p],
with M = B*D*H*W, which equals the reference expression exactly.
"""

import numpy as np

N_CORES = 8
B, C, D, H, W = 2, 16, 64, 128, 128
HW = H * W            # 16384 free elems per (b, partition)
P = 128               # (C=16) x (D_local=8) partitions
D_LOCAL = D // N_CORES


def build_bass_kernel(free=HW, n_b=B, dma_chunk=8192, sub=2048):
    """Build the per-core Bass/Tile kernel.

    Inputs  : pred, true  [n_b, 128, free] f32 (shard, class*d_local on axis 1)
    Output  : out [128, 1] f32 -- per-partition sum of term = t*u + (1-t)*v.
    """
    import concourse.bacc as bacc
    import concourse.mybir as mybir
    import concourse.tile as tile
    from concourse.alu_op_type import AluOpType

    f32 = mybir.dt.float32
    AF = mybir.ActivationFunctionType

    assert free % dma_chunk == 0 and dma_chunk % sub == 0
    n_dma = free // dma_chunk
    n_sub = dma_chunk // sub
    ncols = n_b * n_dma * n_sub

    nc = bacc.Bacc("TRN2", target_bir_lowering=False, debug=False,
                   num_devices=N_CORES)
    pred_d = nc.dram_tensor("pred", [n_b, P, free], f32, kind="ExternalInput")
    true_d = nc.dram_tensor("true", [n_b, P, free], f32, kind="ExternalInput")
    out_d = nc.dram_tensor("out", [P, 1], f32, kind="ExternalOutput")

    with tile.TileContext(nc) as tc:
        with (
            tc.tile_pool(name="pin", bufs=2) as pin,
            tc.tile_pool(name="tin", bufs=2) as tin,
            tc.tile_pool(name="uv", bufs=3) as uvp,
            tc.tile_pool(name="small", bufs=1) as small,
        ):
            bias0 = small.tile([P, 1], f32, tag="bias0")
            bias1 = small.tile([P, 1], f32, tag="bias1")
            nc.vector.memset(bias0[:], 0.0)
            nc.vector.memset(bias1[:], 1.0)
            acc_tu = small.tile([P, ncols], f32, tag="acc_tu")
            acc_w = small.tile([P, ncols], f32, tag="acc_w")

            col = 0
            for b in range(n_b):
                for j in range(n_dma):
                    p_t = pin.tile([P, dma_chunk], f32, tag="p")
                    t_t = tin.tile([P, dma_chunk], f32, tag="t")
                    sl = slice(j * dma_chunk, (j + 1) * dma_chunk)
                    nc.sync.dma_start(p_t[:], pred_d[b, :, sl])
                    nc.sync.dma_start(t_t[:], true_d[b, :, sl])
                    for k in range(n_sub):
                        ss = slice(k * sub, (k + 1) * sub)
                        u = uvp.tile([P, sub], f32, tag="u")
                        v = uvp.tile([P, sub], f32, tag="v")
                        # u = ln(p), v = ln(1 - p)
                        nc.scalar.activation(u[:], p_t[:, ss], AF.Ln,
                                             bias=bias0[:], scale=1.0)
                        nc.scalar.activation(v[:], p_t[:, ss], AF.Ln,
                                             bias=bias1[:], scale=-1.0)
                        # acc_tu[:, col] = sum((t - 0) * u)
                        nc.vector.scalar_tensor_tensor(
                            u[:], t_t[:, ss], 0.0, u[:],
                            AluOpType.subtract, AluOpType.mult,
                            accum_out=acc_tu[:, col:col + 1])
                        # acc_w[:, col] = sum((t - 1) * v)
                        nc.vector.scalar_tensor_tensor(
                            v[:], t_t[:, ss], 1.0, v[:],
                            AluOpType.subtract, AluOpType.mult,
                            accum_out=acc_w[:, col:col + 1])
                        col += 1

            rtu = small.tile([P, 1], f32, tag="rtu")
            rw = small.tile([P, 1], f32, tag="rw")
            total = small.tile([P, 1], f32, tag="total")
            nc.vector.reduce_sum(rtu[:], acc_tu[:], axis=mybir.AxisListType.X)
            nc.vector.reduce_sum(rw[:], acc_w[:], axis=mybir.AxisListType.X)
            nc.vector.tensor_sub(total[:], rtu[:], rw[:])
            nc.sync.dma_start(out_d[:], total[:])

    nc.compile()
    return nc


_NC_CACHE = {}


def _get_nc():
    if "nc" not in _NC_CACHE:
        _NC_CACHE["nc"] = build_bass_kernel()
    return _NC_CACHE["nc"]


def shard_inputs(pred, true):
    """Full [B,C,D,H,W] -> per-core [B, 128, HW] contiguous shards."""
    in_maps = []
    for i in range(N_CORES):
        d0 = i * D_LOCAL
        ps = np.ascontiguousarray(
            pred[:, :, d0:d0 + D_LOCAL].reshape(B, P, HW))
        ts = np.ascontiguousarray(
            true[:, :, d0:d0 + D_LOCAL].reshape(B, P, HW))
        in_maps.append({"pred": ps, "true": ts})
    return in_maps


def combine(per_core_totals, weight):
    """per_core_totals [n_cores, 128]; weight [16] -> scalar f32."""
    w = np.asarray(weight, dtype=np.float64)
    m = float(B * D * H * W)
    wfold = -np.repeat(w, D_LOCAL) / (m * w.sum())          # [128]
    totals = np.asarray(per_core_totals, dtype=np.float64).sum(axis=0)
    return np.float32((wfold * totals).sum())


def kernel(pred, true, weight, _trace=False):
    from concourse.bass_utils import run_bass_kernel_spmd

    nc = _get_nc()
    in_maps = shard_inputs(np.asarray(pred), np.asarray(true))
    res = run_bass_kernel_spmd(nc, in_maps, core_ids=list(range(N_CORES)),
                               trace=_trace)
    totals = np.stack([r["out"][:, 0] for r in res.results])
    out = combine(totals, weight)
    if _trace:
        return out, res
    return out


# revision 4
# speedup vs baseline: 1.1605x; 1.1605x over previous
"""Weighted BCE loss (nn_BCELoss_with_weight) on 8 Trainium2 NeuronCores.

Reference computes:
    log_p   = max(log(pred), -100)            # clamp never binds: pred in [1e-4, 1-1e-4]
    log_1mp = max(log1p(-pred), -100)
    bce     = -(true*log_p + (1-true)*log_1mp)    # [B,C,D,H,W] = [2,16,64,128,128]
    per_class = mean(bce, axes=(0,2,3,4))         # [C]
    out = sum(weight*per_class) / sum(weight)     # scalar

Sharding: D=64 split into 8 slices of 8 (data parallel). Per core the shard
[2,16,8,128,128] is viewed as [B=2, (C,Dl)=128, H*W=16384]: partition p holds
class c=p//8 only, so per-class weighting reduces to a per-partition weighted
sum that the HOST applies to the tiny [128] per-partition totals.

Per core on device (term = t*u + (1-t)*v, u=ln(p), v=ln(1-p)):
    ACT: u = Ln(p);  v = Ln(-p + 1)                       (2 table-evals/elem)
    DVE: tensor_tensor_reduce  -> acc_tu[col] = sum(t*u)  (1 op/elem)
    DVE: scalar_tensor_tensor  -> acc_w[col]  = sum((t-1)*v)
    totals[p] = sum_cols acc_tu - sum_cols acc_w = sum_e term_e
Host: result = sum_{core,p} (-w[p//8]/(M*sum(w))) * totals[core,p],
with M = B*D*H*W, which equals the reference expression exactly.
"""

import numpy as np

N_CORES = 8
B, C, D, H, W = 2, 16, 64, 128, 128
HW = H * W            # 16384 free elems per (b, partition)
P = 128               # (C=16) x (D_local=8) partitions
D_LOCAL = D // N_CORES


def build_bass_kernel(free=HW, n_b=B, dma_chunk=8192, sub=4096,
                      pin_bufs=3, tin_bufs=3, uv_bufs=3, low_dtype=True):
    """Build the per-core Bass/Tile kernel.

    Inputs  : pred, true  [n_b, 128, free] f32 (shard, class*d_local on axis 1)
    Output  : out [128, 1] f32 -- per-partition sum of term = t*u + (1-t)*v.

    low_dtype: keep t/u/v in bf16 so the DVE scalar_tensor_tensor ops hit
    the 2x_1P perf mode (pred stays f32: 1-p in bf16 would bias ln(1-p)
    near p->1; the STT accumulators stay f32).
    """
    import concourse.bacc as bacc
    import concourse.mybir as mybir
    import concourse.tile as tile
    from concourse.alu_op_type import AluOpType

    f32 = mybir.dt.float32
    lowt = mybir.dt.bfloat16 if low_dtype else f32
    AF = mybir.ActivationFunctionType

    assert free % dma_chunk == 0 and dma_chunk % sub == 0
    n_dma = free // dma_chunk
    n_sub = dma_chunk // sub
    ncols = n_b * n_dma * n_sub

    nc = bacc.Bacc("TRN2", target_bir_lowering=False, debug=False,
                   num_devices=N_CORES)
    pred_d = nc.dram_tensor("pred", [n_b, P, free], f32, kind="ExternalInput")
    true_d = nc.dram_tensor("true", [n_b, P, free], f32, kind="ExternalInput")
    out_d = nc.dram_tensor("out", [P, 1], f32, kind="ExternalOutput")

    with tile.TileContext(nc) as tc:
        with (
            tc.tile_pool(name="pin", bufs=pin_bufs) as pin,
            tc.tile_pool(name="tin", bufs=tin_bufs) as tin,
            tc.tile_pool(name="uv", bufs=uv_bufs) as uvp,
            tc.tile_pool(name="small", bufs=1) as small,
        ):
            bias0 = small.tile([P, 1], f32, tag="bias0")
            bias1 = small.tile([P, 1], f32, tag="bias1")
            nc.vector.memset(bias0[:], 0.0)
            nc.vector.memset(bias1[:], 1.0)
            acc_tu = small.tile([P, ncols], f32, tag="acc_tu")
            acc_w = small.tile([P, ncols], f32, tag="acc_w")

            col = 0
            for b in range(n_b):
                for j in range(n_dma):
                    p_t = pin.tile([P, dma_chunk], f32, tag="p")
                    t_t = tin.tile([P, dma_chunk], lowt, tag="t")
                    sl = slice(j * dma_chunk, (j + 1) * dma_chunk)
                    nc.sync.dma_start(p_t[:], pred_d[b, :, sl])
                    if low_dtype:
                        # cast f32 -> bf16 inline (SWDGE-only feature)
                        nc.gpsimd.dma_start(t_t[:], true_d[b, :, sl])
                    else:
                        nc.sync.dma_start(t_t[:], true_d[b, :, sl])
                    for k in range(n_sub):
                        ss = slice(k * sub, (k + 1) * sub)
                        u = uvp.tile([P, sub], lowt, tag="u")
                        v = uvp.tile([P, sub], lowt, tag="v")
                        # u = ln(p), v = ln(1 - p)
                        nc.scalar.activation(u[:], p_t[:, ss], AF.Ln,
                                             bias=bias0[:], scale=1.0)
                        nc.scalar.activation(v[:], p_t[:, ss], AF.Ln,
                                             bias=bias1[:], scale=-1.0)
                        # acc_tu[:, col] = sum((t - 0) * u)
                        nc.vector.scalar_tensor_tensor(
                            u[:], t_t[:, ss], 0.0, u[:],
                            AluOpType.subtract, AluOpType.mult,
                            accum_out=acc_tu[:, col:col + 1])
                        # acc_w[:, col] = sum((t - 1) * v)
                        nc.vector.scalar_tensor_tensor(
                            v[:], t_t[:, ss], 1.0, v[:],
                            AluOpType.subtract, AluOpType.mult,
                            accum_out=acc_w[:, col:col + 1])
                        col += 1

            rtu = small.tile([P, 1], f32, tag="rtu")
            rw = small.tile([P, 1], f32, tag="rw")
            total = small.tile([P, 1], f32, tag="total")
            nc.vector.reduce_sum(rtu[:], acc_tu[:], axis=mybir.AxisListType.X)
            nc.vector.reduce_sum(rw[:], acc_w[:], axis=mybir.AxisListType.X)
            nc.vector.tensor_sub(total[:], rtu[:], rw[:])
            nc.sync.dma_start(out_d[:], total[:])

    nc.compile()
    return nc


_NC_CACHE = {}


def _get_nc():
    if "nc" not in _NC_CACHE:
        _NC_CACHE["nc"] = build_bass_kernel()
    return _NC_CACHE["nc"]


def shard_inputs(pred, true):
    """Full [B,C,D,H,W] -> per-core [B, 128, HW] contiguous shards."""
    in_maps = []
    for i in range(N_CORES):
        d0 = i * D_LOCAL
        ps = np.ascontiguousarray(
            pred[:, :, d0:d0 + D_LOCAL].reshape(B, P, HW))
        ts = np.ascontiguousarray(
            true[:, :, d0:d0 + D_LOCAL].reshape(B, P, HW))
        in_maps.append({"pred": ps, "true": ts})
    return in_maps


def combine(per_core_totals, weight):
    """per_core_totals [n_cores, 128]; weight [16] -> scalar f32."""
    w = np.asarray(weight, dtype=np.float64)
    m = float(B * D * H * W)
    wfold = -np.repeat(w, D_LOCAL) / (m * w.sum())          # [128]
    totals = np.asarray(per_core_totals, dtype=np.float64).sum(axis=0)
    return np.float32((wfold * totals).sum())


def kernel(pred, true, weight, _trace=False):
    from concourse.bass_utils import run_bass_kernel_spmd

    nc = _get_nc()
    in_maps = shard_inputs(np.asarray(pred), np.asarray(true))
    res = run_bass_kernel_spmd(nc, in_maps, core_ids=list(range(N_CORES)),
                               trace=_trace)
    totals = np.stack([r["out"][:, 0] for r in res.results])
    out = combine(totals, weight)
    if _trace:
        return out, res
    return out


# revision 5
# speedup vs baseline: 1.1676x; 1.0061x over previous
"""Weighted BCE loss (nn_BCELoss_with_weight) on 8 Trainium2 NeuronCores.

Reference computes:
    log_p   = max(log(pred), -100)            # clamp never binds: pred in [1e-4, 1-1e-4]
    log_1mp = max(log1p(-pred), -100)
    bce     = -(true*log_p + (1-true)*log_1mp)    # [B,C,D,H,W] = [2,16,64,128,128]
    per_class = mean(bce, axes=(0,2,3,4))         # [C]
    out = sum(weight*per_class) / sum(weight)     # scalar

Sharding: D=64 split into 8 slices of 8 (data parallel). Per core the shard
[2,16,8,128,128] is viewed as [B=2, (C,Dl)=128, H*W=16384]: partition p holds
class c=p//8 only, so the per-class weight is a per-partition scalar.

Per core on device, with u=ln(p), v=ln(1-p), w~=bf16(weight):
    term = t*u + (1-t)*v = t*(u-v) + v
    ACT : u = Ln(p) [bf16 out];  v = Ln(-p+1) [bf16 out, accum_out -> sum(v)]
    DVE : d = u - v (bf16 TT, 2x);  m = t*d (bf16 TT, 2x)   [t cast via SWDGE DMA]
    PE  : psum[1,512] += w~[128,1].T @ m[:,512-chunk]  (f32 accumulate)
    out_m[1,1]  = sum(psum)           -- already class-weighted
    out_v[128,1] = per-partition sum(v)
Host: result = -(sum_cores out_m + sum_p w~[p//8]*out_v[p]) / (M*sum(w~)),
with M = B*D*H*W.  Using the bf16-rounded weights consistently in both the
numerator and denominator makes this the exact weighted-mean of per-class BCE
with weights w~; since per-class means are ~equal, the w->w~ rounding
perturbs the result by ~|delta_w|*spread(per_class) ~ 1e-5 relative.
"""

import numpy as np

N_CORES = 8
B, C, D, H, W = 2, 16, 64, 128, 128
HW = H * W            # 16384 free elems per (b, partition)
P = 128               # (C=16) x (D_local=8) partitions
D_LOCAL = D // N_CORES
MM_N = 512            # one PSUM bank of f32


def build_bass_kernel(free=HW, n_b=B, dma_chunk=8192, sub=4096,
                      pin_bufs=3, tin_bufs=3, uv_bufs=3):
    """Build the per-core Bass/Tile kernel.

    Inputs  : pred, true [n_b, 128, free] f32 (shard, class*d_local on axis 1)
              wf [128, 1] bf16 (per-partition class weight)
    Outputs : out_m [1, 1] f32   = sum_p wf[p] * sum_e (t*(u-v))[p, e]
              out_v [128, 1] f32 = per-partition sum_e v[p, e]
    """
    import concourse.bacc as bacc
    import concourse.mybir as mybir
    import concourse.tile as tile
    from concourse.alu_op_type import AluOpType

    f32 = mybir.dt.float32
    bf16 = mybir.dt.bfloat16
    AF = mybir.ActivationFunctionType

    assert free % dma_chunk == 0 and dma_chunk % sub == 0 and sub % MM_N == 0
    n_dma = free // dma_chunk
    n_sub = dma_chunk // sub
    ncols = n_b * n_dma * n_sub
    n_mm = sub // MM_N
    total_mm = ncols * n_mm

    nc = bacc.Bacc("TRN2", target_bir_lowering=False, debug=False,
                   num_devices=N_CORES)
    pred_d = nc.dram_tensor("pred", [n_b, P, free], f32, kind="ExternalInput")
    true_d = nc.dram_tensor("true", [n_b, P, free], f32, kind="ExternalInput")
    wf_d = nc.dram_tensor("wf", [P, 1], bf16, kind="ExternalInput")
    outm_d = nc.dram_tensor("out_m", [1, 1], f32, kind="ExternalOutput")
    outv_d = nc.dram_tensor("out_v", [P, 1], f32, kind="ExternalOutput")

    with tile.TileContext(nc) as tc:
        with (
            tc.tile_pool(name="pin", bufs=pin_bufs) as pin,
            tc.tile_pool(name="tin", bufs=tin_bufs) as tin,
            tc.tile_pool(name="uv", bufs=uv_bufs) as uvp,
            tc.tile_pool(name="small", bufs=1) as small,
            tc.tile_pool(name="psum", bufs=1, space="PSUM") as psump,
        ):
            bias0 = small.tile([P, 1], f32, tag="bias0")
            bias1 = small.tile([P, 1], f32, tag="bias1")
            nc.vector.memset(bias0[:], 0.0)
            nc.vector.memset(bias1[:], 1.0)
            wf_t = small.tile([P, 1], bf16, tag="wf")
            nc.sync.dma_start(wf_t[:], wf_d[:])
            vacc = small.tile([P, ncols], f32, tag="vacc")
            acc_m = psump.tile([1, MM_N], f32, tag="acc_m")

            col = 0
            mm_i = 0
            for b in range(n_b):
                for j in range(n_dma):
                    p_t = pin.tile([P, dma_chunk], f32, tag="p")
                    t_t = tin.tile([P, dma_chunk], bf16, tag="t")
                    sl = slice(j * dma_chunk, (j + 1) * dma_chunk)
                    nc.sync.dma_start(p_t[:], pred_d[b, :, sl])
                    # f32 -> bf16 cast inline (SWDGE-only feature)
                    nc.gpsimd.dma_start(t_t[:], true_d[b, :, sl])
                    for k in range(n_sub):
                        ss = slice(k * sub, (k + 1) * sub)
                        u = uvp.tile([P, sub], bf16, tag="u")
                        v = uvp.tile([P, sub], bf16, tag="v")
                        # u = ln(p); v = ln(1 - p), vacc[:, col] = sum(v)
                        nc.scalar.activation(u[:], p_t[:, ss], AF.Ln,
                                             bias=bias0[:], scale=1.0)
                        nc.scalar.activation(v[:], p_t[:, ss], AF.Ln,
                                             bias=bias1[:], scale=-1.0,
                                             accum_out=vacc[:, col:col + 1])
                        # u <- d = u - v ; u <- m = t * d   (bf16 2x TT)
                        nc.vector.tensor_sub(u[:], u[:], v[:])
                        nc.vector.tensor_mul(u[:], t_t[:, ss], u[:])
                        # acc_m[1, 512] += wf.T @ m[:, 512-chunk]
                        for q in range(n_mm):
                            nc.tensor.matmul(
                                acc_m[:],
                                wf_t[:],
                                u[:, q * MM_N:(q + 1) * MM_N],
                                start=(mm_i == 0),
                                stop=(mm_i == total_mm - 1),
                            )
                            mm_i += 1
                        col += 1

            outv_t = small.tile([P, 1], f32, tag="outv")
            nc.vector.reduce_sum(outv_t[:], vacc[:], axis=mybir.AxisListType.X)
            nc.sync.dma_start(outv_d[:], outv_t[:])
            accm_sb = small.tile([1, MM_N], f32, tag="accm_sb")
            nc.vector.tensor_copy(accm_sb[:], acc_m[:])
            outm_t = small.tile([1, 1], f32, tag="outm")
            nc.vector.reduce_sum(outm_t[:], accm_sb[:], axis=mybir.AxisListType.X)
            nc.sync.dma_start(outm_d[:], outm_t[:])

    nc.compile()
    return nc


_NC_CACHE = {}


def _get_nc():
    if "nc" not in _NC_CACHE:
        _NC_CACHE["nc"] = build_bass_kernel()
    return _NC_CACHE["nc"]


def _bf16_round(x):
    """Round f32 array to bf16 values (kept in f32 representation)."""
    xi = np.asarray(x, dtype=np.float32).view(np.uint32)
    rounded = ((xi + 0x7FFF + ((xi >> 16) & 1)) & 0xFFFF0000).astype(np.uint32)
    return rounded.view(np.float32)


def shard_inputs(pred, true, weight):
    """Full [B,C,D,H,W] -> per-core in_maps."""
    import ml_dtypes

    wtile = np.repeat(np.asarray(weight, np.float32), D_LOCAL).reshape(P, 1)
    wf = wtile.astype(ml_dtypes.bfloat16)
    in_maps = []
    for i in range(N_CORES):
        d0 = i * D_LOCAL
        ps = np.ascontiguousarray(
            pred[:, :, d0:d0 + D_LOCAL].reshape(B, P, HW))
        ts = np.ascontiguousarray(
            true[:, :, d0:d0 + D_LOCAL].reshape(B, P, HW))
        in_maps.append({"pred": ps, "true": ts, "wf": wf})
    return in_maps


def combine(out_ms, out_vs, weight):
    """out_ms [n_cores] scalars, out_vs [n_cores, 128]; weight [16] f32."""
    wt = _bf16_round(np.repeat(np.asarray(weight, np.float32), D_LOCAL))
    wt64 = wt.astype(np.float64)
    m = float(B * D * H * W)
    w_sum = wt64[::D_LOCAL].sum()          # sum of the 16 bf16 class weights
    total_v = (np.asarray(out_vs, np.float64).sum(axis=0) * wt64).sum()
    total_m = float(np.asarray(out_ms, np.float64).sum())
    return np.float32(-(total_m + total_v) / (m * w_sum))


def kernel(pred, true, weight, _trace=False):
    from concourse.bass_utils import run_bass_kernel_spmd

    nc = _get_nc()
    in_maps = shard_inputs(np.asarray(pred), np.asarray(true), weight)
    res = run_bass_kernel_spmd(nc, in_maps, core_ids=list(range(N_CORES)),
                               trace=_trace)
    out_ms = [r["out_m"][0, 0] for r in res.results]
    out_vs = [r["out_v"][:, 0] for r in res.results]
    out = combine(out_ms, out_vs, weight)
    if _trace:
        return out, res
    return out
